# revision 11
# baseline (speedup 1.0000x reference)
"""Trainium2 Bass kernel for nn_MidBlock (resnet -> attention -> resnet).

Sharding: 8 cores = (batch b, H-half h); core c handles batch c//2, rows
32*(c%2) .. +32.  GroupNorm stats pair-AllReduced; attention k/vT pair
AllGathered; conv halos computed redundantly from a host-padded input and a
boundary-row exchange before the second resnet.

Matmul dtype: float32r (1 cycle/row at N>=256, ~tf32 precision).
"""
import sys
sys.path.insert(0, '/opt/trn_rl_repo')
import numpy as np

import concourse.bass as bass
import concourse.bacc as bacc
import concourse.tile as tile
import concourse.mybir as mybir
from concourse.bass_utils import run_bass_kernel_spmd

f32 = mybir.dt.float32
f32r = mybir.dt.float32r
AF = mybir.ActivationFunctionType
ALU = mybir.AluOpType

NCORES = 8
PAIRS = [[0, 1], [2, 3], [4, 5], [6, 7]]
C = 512
CT = 4          # channel tiles of 128
G = 32          # groups
W = 64
WP = 66
HS = 32         # owned rows per core
NTOK = HS * W   # 2048 local tokens
NLOC_JT = NTOK // 128   # 16
EPS = 1e-5
GN_N = 16 * 64 * 64     # elements per group per batch
ROWS34 = [(0, 7), (7, 7), (14, 7), (21, 7), (28, 6)]
ROWS32 = [(0, 8), (8, 8), (16, 8), (24, 8)]
_KG = [None]


def build_midblock(num_devices=NCORES, collectives=True, reps=1, debug_outs=False):
    nc = bacc.Bacc("TRN2", target_bir_lowering=False, debug=False,
                   num_devices=num_devices)
    pairs = PAIRS if collectives else None

    xpad_d = nc.dram_tensor("xpad", [C, 36, W], f32, kind="ExternalInput")
    cw_d = nc.dram_tensor("cw", [4, CT, 128, 9, CT, 128], f32, kind="ExternalInput")
    cb_d = nc.dram_tensor("cb", [4, CT, 128], f32, kind="ExternalInput")
    gn_d = nc.dram_tensor("gn", [5, CT, 128, 2], f32, kind="ExternalInput")
    wqko_d = nc.dram_tensor("wqko", [3, CT, 128, CT * 128], f32, kind="ExternalInput")
    wv_d = nc.dram_tensor("wv", [CT, 128, C], f32, kind="ExternalInput")
    ab_d = nc.dram_tensor("ab", [3, CT, 128], f32, kind="ExternalInput")
    bvb_d = nc.dram_tensor("bvb", [128, C], f32, kind="ExternalInput")
    gmask_d = nc.dram_tensor("gmask", [CT, 128, G], f32, kind="ExternalInput")
    bmask_d = nc.dram_tensor("bmask", [CT, G, 128], f32, kind="ExternalInput")
    pm_d = nc.dram_tensor("pm", [128, 2], f32, kind="ExternalInput")
    y_d = nc.dram_tensor("y", [C, HS, W], f32, kind="ExternalOutput")
    dbg = None
    if debug_outs:
        dbg = {"y1": nc.dram_tensor("y1", [CT, 128, NTOK], f32, kind="ExternalOutput"),
               "y2": nc.dram_tensor("y2", [CT, 128, NTOK], f32, kind="ExternalOutput")}
        if debug_outs > 1:
            dbg["yq"] = nc.dram_tensor("yq", [CT, 128, NTOK], f32, kind="ExternalOutput")
            dbg["yk"] = nc.dram_tensor("yk", [CT, 128, NTOK], f32, kind="ExternalOutput")
            dbg["yv"] = nc.dram_tensor("yv", [NLOC_JT, 128, 520], f32, kind="ExternalOutput")
            dbg["ykg"] = nc.dram_tensor("ykg", [2, CT, 128, NTOK], f32, kind="ExternalOutput")
            dbg["yet"] = nc.dram_tensor("yet", [2, 128, 512], f32, kind="ExternalOutput")
            dbg["yden"] = nc.dram_tensor("yden", [4, 1, 512], f32, kind="ExternalOutput")
            dbg["yhn"] = nc.dram_tensor("yhn", [CT, 128, NTOK], f32, kind="ExternalOutput")
            dbg["yab"] = nc.dram_tensor("yab", [CT, 128, 2], f32, kind="ExternalOutput")

    with tile.TileContext(nc) as tc:
        with tc.tile_pool(name="pg", bufs=1) as pg, \
             tc.tile_pool(name="pp", bufs=1, space="PSUM") as pp, \
             tc.tile_pool(name="pd", bufs=1, space="DRAM") as pd:
            for rep in range(reps):
                _body(nc, tc, pg, pp, pd, pairs, rep,
                      xpad_d, cw_d, cb_d, gn_d, wqko_d, wv_d, ab_d, bvb_d,
                      gmask_d, bmask_d, pm_d, y_d, dbg)
    nc.compile()
    return nc


def _body(nc, tc, pg, pp, pd, pairs, rep,
          xpad_d, cw_d, cb_d, gn_d, wqko_d, wv_d, ab_d, bvb_d,
          gmask_d, bmask_d, pm_d, y_d, dbg=None):
    R = f"r{rep}"

    # ---------- global small tiles ----------
    gmask = []
    bmask = []
    for ct in range(CT):
        gm = pg.tile([128, G], f32, tag="gmask", bufs=CT, name=f"gm{R}_{ct}")
        nc.sync.dma_start(gm[:], gmask_d[ct, :, :])
        gmask.append(gm)
        bm = pg.tile([G, 128], f32, tag="bmask", bufs=CT, name=f"bm{R}_{ct}")
        nc.sync.dma_start(bm[:], bmask_d[ct, :, :])
        bmask.append(bm)
    pm = pg.tile([128, 2], f32, tag="pm", bufs=1, name=f"pm{R}")
    nc.sync.dma_start(pm[:], pm_d[:, :])
    ones_r = pg.tile([1, 128], f32, tag="ones_r", bufs=1, name=f"onr{R}")
    nc.vector.memset(ones_r[:], 1.0)

    def load_bias(pool, src_ap, tagn):
        out = []
        for ct in range(CT):
            b = pool.tile([128, 1], f32, tag=tagn, bufs=CT, name=f"{tagn}{R}_{ct}")
            nc.sync.dma_start(b[:], src_ap[ct, :])
            out.append(b)
        return out

    # preallocated per-channel (A, B) scale/bias tiles for all 5 GNs, so the
    # global pool never grows while a phase pool is open (fragmentation)
    AB = []
    for gi in range(5):
        Al = [pg.tile([128, 1], f32, tag=f"g{gi}A", bufs=CT, name=f"A{R}_{gi}_{c}")
              for c in range(CT)]
        Bl = [pg.tile([128, 1], f32, tag=f"g{gi}B", bufs=CT, name=f"B{R}_{gi}_{c}")
              for c in range(CT)]
        AB.append((Al, Bl))

    def gn_finalize(pool, gn_idx, stats2):
        """stats2: CT tiles [128,2] (sum, sumsq) per channel partition.
        Returns per-channel (A, B) scale/bias tiles [128,1] f32."""
        ps_g = pp.tile([G, 2], f32, tag="sc", bufs=3, name=f"psg{R}_{gn_idx}")
        for ct in range(CT):
            nc.tensor.matmul(ps_g[:], gmask[ct][:], stats2[ct][:],
                             start=(ct == 0), stop=(ct == CT - 1))
        sg = pool.tile([G, 2], f32, tag="sg", bufs=2, name=f"sg{R}_{gn_idx}")
        nc.scalar.activation(sg[:], ps_g[:], AF.Copy)
        sg2 = pool.tile([G, 2], f32, tag="sg2", bufs=2, name=f"sg2{R}_{gn_idx}")
        if pairs is not None:
            st_in = pd.tile([G, 2], f32, tag="st_in", bufs=2, name=f"sti{R}_{gn_idx}")
            st_out = pd.tile([G, 2], f32, tag="st_out", bufs=2, name=f"sto{R}_{gn_idx}")
            nc.sync.dma_start(st_in[:], sg[:])
            nc.gpsimd.collective_compute(
                "AllReduce", ALU.add, replica_groups=pairs,
                ins=[st_in[:].opt()], outs=[st_out[:].opt()])
            nc.sync.dma_start(sg2[:], st_out[:])
        else:
            nc.vector.tensor_copy(sg2[:], sg[:])
        mean2 = pool.tile([G, 2], f32, tag="mean2", bufs=2, name=f"mn{R}_{gn_idx}")
        nc.vector.tensor_scalar_mul(mean2[:], sg2[:], 1.0 / GN_N)
        var = pool.tile([G, 1], f32, tag="var", bufs=2, name=f"var{R}_{gn_idx}")
        nc.vector.scalar_tensor_tensor(var[:], mean2[:, 0:1], 1.0, mean2[:, 0:1],
                                       op0=ALU.mult, op1=ALU.mult)
        nc.vector.tensor_tensor(var[:], mean2[:, 1:2], var[:], op=ALU.subtract)
        nc.vector.tensor_scalar_add(var[:], var[:], EPS)
        sd = pool.tile([G, 1], f32, tag="sd", bufs=2, name=f"sd{R}_{gn_idx}")
        nc.scalar.activation(sd[:], var[:], AF.Sqrt)
        grp2 = pool.tile([G, 2], f32, tag="grp2", bufs=2, name=f"grp{R}_{gn_idx}")
        nc.vector.reciprocal(grp2[:, 0:1], sd[:])
        nc.vector.tensor_tensor(grp2[:, 1:2], mean2[:, 0:1], grp2[:, 0:1], op=ALU.mult)
        nc.vector.tensor_scalar_mul(grp2[:, 1:2], grp2[:, 1:2], -1.0)
        A, B = AB[gn_idx]
        for ct in range(CT):
            gnp = pool.tile([128, 2], f32, tag="gnp", bufs=2 * CT,
                            name=f"gnp{R}_{gn_idx}_{ct}")
            nc.sync.dma_start(gnp[:], gn_d[gn_idx, ct, :, :])
            ps_b = pp.tile([128, 2], f32, tag="sc", bufs=3, name=f"psb{R}_{gn_idx}_{ct}")
            nc.tensor.matmul(ps_b[:], bmask[ct][:], grp2[:], start=True, stop=True)
            bc = pool.tile([128, 2], f32, tag="bc", bufs=2 * CT,
                           name=f"bc{R}_{gn_idx}_{ct}")
            nc.scalar.activation(bc[:], ps_b[:], AF.Copy)
            nc.vector.tensor_tensor(A[ct][:], gnp[:, 0:1], bc[:, 0:1], op=ALU.mult)
            nc.vector.scalar_tensor_tensor(B[ct][:], gnp[:, 0:1], 1.0, bc[:, 1:2],
                                           op0=ALU.mult, op1=ALU.mult)
            nc.vector.tensor_tensor(B[ct][:], B[ct][:], gnp[:, 1:2], op=ALU.add)
        return A, B

    def conv3x3(pool, hbuf, nrows_out, rowplan, acc_tag, conv_idx, cb_tiles,
                epilogue):
        acc = [pool.tile([128, 34 * W], f32, tag=acc_tag, bufs=CT,
                         name=f"acc{R}_{conv_idx}_{ct}") for ct in range(CT)]
        for cit in range(CT):
            wt = pool.tile([128, 9 * C], f32r, tag="cw", bufs=2,
                           name=f"cw{R}_{conv_idx}_{cit}")
            nc.sync.dma_start(wt[:], cw_d[conv_idx, cit, :, :, :, :].bitcast(f32r))
            hb = hbuf[cit][:].rearrange("p (h w) -> p h w", w=WP)
            for cot in range(CT):
                for (r0, Rr) in rowplan:
                    ps = pp.tile([128, 512], f32, tag="acc_ps", bufs=4,
                                 name=f"cps{R}_{conv_idx}_{cit}_{cot}_{r0}")
                    psv = ps[:, 0:Rr * W]
                    for tap in range(9):
                        dy, dx = tap // 3, tap % 3
                        rhs = hb[:, r0 + dy:r0 + dy + Rr, dx:dx + W]
                        nc.tensor.matmul(
                            psv, wt[:, tap * C + cot * 128:tap * C + cot * 128 + 128],
                            rhs, start=(tap == 0), stop=(tap == 8))
                    accv = acc[cot][:, r0 * W:(r0 + Rr) * W]
                    if cit == 0:
                        nc.scalar.activation(accv, psv, AF.Identity, bias=cb_tiles[cot][:])
                    else:
                        nc.vector.tensor_tensor(accv, accv, psv, op=ALU.add)
                    if cit == CT - 1 and epilogue is not None:
                        epilogue(cot, r0, Rr, acc[cot])
        return acc

    def stats_cols(pool, tagn, ncols):
        return [pool.tile([128, ncols], f32, tag=tagn, bufs=CT,
                          name=f"{tagn}{R}_{ct}") for ct in range(CT)]

    def reduce_stats(pool, sumc, sqc, tagn):
        out = []
        for ct in range(CT):
            s2 = pool.tile([128, 2], f32, tag=tagn, bufs=CT, name=f"{tagn}{R}_{ct}")
            nc.vector.reduce_sum(s2[:, 0:1], sumc[ct][:], axis=mybir.AxisListType.X)
            nc.vector.reduce_sum(s2[:, 1:2], sqc[ct][:], axis=mybir.AxisListType.X)
            out.append(s2)
        return out

    # DRAM spill tensors
    x1_dram = pd.tile([CT, 128, NTOK], f32, tag="x1d", bufs=1, name=f"x1d{R}")
    x2_dram = pd.tile([CT, 128, NTOK], f32, tag="x2d", bufs=1, name=f"x2d{R}")

    # =================== RESNET 1 ===================
    with tc.tile_pool(name=f"p1{R}", bufs=1) as p1:
        cb0 = load_bias(p1, cb_d[0], "cb0")
        cb1 = load_bias(p1, cb_d[1], "cb1")
        scr = [p1.tile([128, HS * W], f32, tag="scr", bufs=2, name=f"scr{R}_{i}")
               for i in range(2)]
        xp = []
        s1sum = stats_cols(p1, "s1sum", 1)
        s1sq = stats_cols(p1, "s1sq", 1)
        for ct in range(CT):
            x = p1.tile([128, 36 * W], f32, tag="xp", bufs=CT, name=f"xp{R}_{ct}")
            nc.sync.dma_start(x[:], xpad_d[ct * 128:(ct + 1) * 128, :, :])
            xp.append(x)
            own = x[:, 2 * W:34 * W]
            nc.scalar.activation(scr[0][:], own, AF.Copy, accum_out=s1sum[ct][:, 0:1])
            nc.vector.scalar_tensor_tensor(scr[1][:], own, 1.0, own, op0=ALU.mult,
                                           op1=ALU.mult, accum_out=s1sq[ct][:, 0:1])
        st1 = reduce_stats(p1, s1sum, s1sq, "st1")
        A1, B1 = gn_finalize(p1, 0, st1)

        h1 = []
        for ct in range(CT):
            h = p1.tile([128, 36 * WP], f32r, tag="hp", bufs=CT, name=f"h1{R}_{ct}")
            hr = h[:].rearrange("p (h w) -> p h w", w=WP)
            srcb = xp[ct][:, 0:36].rearrange("p (a b) -> p a b", a=36)
            nc.vector.tensor_scalar_mul(hr[:, :, 0:1], srcb, 0.0)
            nc.vector.tensor_scalar_mul(hr[:, :, WP - 1:WP], srcb, 0.0)
            xv = xp[ct][:].rearrange("p (h w) -> p h w", w=W)
            nc.scalar.activation(hr[:, :, 1:WP - 1], xv, AF.Silu,
                                 bias=B1[ct][:], scale=A1[ct][:])
            # zero image-boundary halo rows (top unless odd core, bottom unless even)
            nc.vector.tensor_scalar_mul(hr[:, 0:2, 1:WP - 1], hr[:, 0:2, 1:WP - 1],
                                        pm[:, 0:1])
            nc.vector.tensor_scalar_mul(hr[:, 34:36, 1:WP - 1], hr[:, 34:36, 1:WP - 1],
                                        pm[:, 1:2])
            h1.append(h)

        s2sum = stats_cols(p1, "s2sum", len(ROWS34))
        s2sq = stats_cols(p1, "s2sq", len(ROWS34))

        def ep1(cot, r0, Rr, accT):
            a0, a1 = max(r0, 1), min(r0 + Rr, 33)   # owned rows of the 34-row grid
            v = accT[:, a0 * W:a1 * W]
            ci = ROWS34.index((r0, Rr))
            nc.scalar.activation(scr[0][:, 0:(a1 - a0) * W], v, AF.Copy,
                                 accum_out=s2sum[cot][:, ci:ci + 1])
            nc.vector.scalar_tensor_tensor(scr[1][:, 0:(a1 - a0) * W], v, 1.0, v,
                                           op0=ALU.mult, op1=ALU.mult,
                                           accum_out=s2sq[cot][:, ci:ci + 1])

        o1 = conv3x3(p1, h1, 34, ROWS34, "accA", 0, cb0, ep1)
        st2 = reduce_stats(p1, s2sum, s2sq, "st2")
        A2, B2 = gn_finalize(p1, 1, st2)

        h2 = []
        for ct in range(CT):
            h = p1.tile([128, 34 * WP], f32r, tag="hp", bufs=CT, name=f"h2{R}_{ct}")
            hr = h[:].rearrange("p (h w) -> p h w", w=WP)
            srcb = xp[ct][:, 0:34].rearrange("p (a b) -> p a b", a=34)
            nc.vector.tensor_scalar_mul(hr[:, :, 0:1], srcb, 0.0)
            nc.vector.tensor_scalar_mul(hr[:, :, WP - 1:WP], srcb, 0.0)
            ov = o1[ct][:].rearrange("p (h w) -> p h w", w=W)
            nc.scalar.activation(hr[:, :, 1:WP - 1], ov, AF.Silu,
                                 bias=B2[ct][:], scale=A2[ct][:])
            nc.vector.tensor_scalar_mul(hr[:, 0:1, 1:WP - 1], hr[:, 0:1, 1:WP - 1],
                                        pm[:, 0:1])
            nc.vector.tensor_scalar_mul(hr[:, 33:34, 1:WP - 1], hr[:, 33:34, 1:WP - 1],
                                        pm[:, 1:2])
            h2.append(h)

        s3sum = stats_cols(p1, "s3sum", len(ROWS32))
        s3sq = stats_cols(p1, "s3sq", len(ROWS32))

        def ep2(cot, r0, Rr, accT):
            ci = ROWS32.index((r0, Rr))
            v = accT[:, r0 * W:(r0 + Rr) * W]
            xv = xp[cot][:, (2 + r0) * W:(2 + r0 + Rr) * W]
            nc.vector.scalar_tensor_tensor(v, v, 1.0, xv, op0=ALU.mult, op1=ALU.add,
                                           accum_out=s3sum[cot][:, ci:ci + 1])
            nc.vector.scalar_tensor_tensor(scr[1][:, 0:Rr * W], v, 1.0, v,
                                           op0=ALU.mult, op1=ALU.mult,
                                           accum_out=s3sq[cot][:, ci:ci + 1])
            nc.sync.dma_start(x1_dram[cot, :, r0 * W:(r0 + Rr) * W], v)

        conv3x3(p1, h2, HS, ROWS32, "accA", 1, cb1, ep2)
        st3 = reduce_stats(p1, s3sum, s3sq, "st3")
        A3, B3 = gn_finalize(p1, 2, st3)

    # =================== ATTENTION ===================
    with tc.tile_pool(name=f"pa{R}", bufs=1) as pa:
        abq = load_bias(pa, ab_d[0], "abq")
        abk = load_bias(pa, ab_d[1], "abk")
        abo = load_bias(pa, ab_d[2], "abo")
        bvb = pa.tile([128, C], f32, tag="bvb", bufs=1, name=f"bvb{R}")
        nc.sync.dma_start(bvb[:], bvb_d[:, :])
        wo = []
        for ct in range(CT):
            wt = pa.tile([128, C], f32r, tag="wo", bufs=CT, name=f"wo{R}_{ct}")
            nc.sync.dma_start(wt[:], wqko_d[2, ct, :, :].bitcast(f32r))
            wo.append(wt)
        q = [pa.tile([128, NTOK], f32r, tag="q", bufs=CT, name=f"q{R}_{c}")
             for c in range(CT)]
        k = [pa.tile([128, NTOK], f32r, tag="k", bufs=CT, name=f"k{R}_{c}")
             for c in range(CT)]
        vT = [pa.tile([128, 520], f32r, tag="vt", bufs=NLOC_JT, name=f"vt{R}_{jt}")
              for jt in range(NLOC_JT)]

        with tc.tile_pool(name=f"pre{R}", bufs=1) as pre:
            hn = []
            for ct in range(CT):
                xt = pre.tile([128, NTOK], f32, tag="xt", bufs=1, name=f"xt{R}_{ct}")
                nc.sync.dma_start(xt[:], x1_dram[ct, :, :])
                h = pre.tile([128, NTOK], f32r, tag="hn", bufs=CT, name=f"hn{R}_{ct}")
                # NOTE: the attention block's GroupNorm has NO activation
                nc.scalar.activation(h[:], xt[:], AF.Identity,
                                     bias=B3[ct][:], scale=A3[ct][:])
                hn.append(h)
                if dbg is not None and "yhn" in dbg:
                    nc.sync.dma_start(dbg["yhn"][ct, :, :], h[:].bitcast(f32))
                    nc.sync.dma_start(dbg["yab"][ct, :, 0:1], A3[ct][:])
                    nc.sync.dma_start(dbg["yab"][ct, :, 1:2], B3[ct][:])

            for idx, (lst, bias) in enumerate(((q, abq), (k, abk))):
                wts = []
                for ct in range(CT):
                    wt = pre.tile([128, C], f32r, tag="aw", bufs=CT,
                                  name=f"aw{R}_{idx}_{ct}")
                    nc.sync.dma_start(wt[:], wqko_d[idx, ct, :, :].bitcast(f32r))
                    wts.append(wt)
                for cot in range(CT):
                    for icn in range(4):
                        sl = slice(icn * 512, (icn + 1) * 512)
                        ps = pp.tile([128, 512], f32, tag="sc", bufs=3,
                                     name=f"psqk{R}_{idx}_{cot}_{icn}")
                        for cit in range(CT):
                            nc.tensor.matmul(
                                ps[:], wts[cit][:, cot * 128:cot * 128 + 128],
                                hn[cit][:, sl],
                                start=(cit == 0), stop=(cit == CT - 1))
                        nc.scalar.activation(lst[cot][:, sl], ps[:], AF.Identity,
                                             bias=bias[cot][:])

            wv = []
            for ct in range(CT):
                wt = pre.tile([128, C], f32r, tag="aw", bufs=CT,
                              name=f"awv{R}_{ct}")
                nc.sync.dma_start(wt[:], wv_d[ct, :, :].bitcast(f32r))
                wv.append(wt)
            for jt in range(NLOC_JT):
                ps = pp.tile([128, 512], f32, tag="sc", bufs=3, name=f"psv{R}_{jt}")
                for cit in range(CT):
                    nc.tensor.matmul(ps[:], hn[cit][:, jt * 128:(jt + 1) * 128],
                                     wv[cit][:], start=(cit == 0), stop=(cit == CT - 1))
                t = vT[jt]
                nc.scalar.activation(t[:, 0:C], ps[:], AF.Copy)
                nc.vector.tensor_tensor(t[:, 0:C], t[:, 0:C], bvb[:], op=ALU.add)
                nc.vector.tensor_scalar(t[:, C:C + 1], bvb[:, 0:1], 0.0, 1.0,
                                        op0=ALU.mult, op1=ALU.add)

        if dbg is not None and "yq" in dbg:
            for ct in range(CT):
                nc.sync.dma_start(dbg["yq"][ct, :, :], q[ct][:].bitcast(f32))
                nc.sync.dma_start(dbg["yk"][ct, :, :], k[ct][:].bitcast(f32))
            for jt in range(NLOC_JT):
                nc.sync.dma_start(dbg["yv"][jt, :, :], vT[jt][:].bitcast(f32))
        # exchange k / vT with pair partner
        if pairs is not None:
            kb = pd.tile([CT, 128, NTOK], f32, tag="kb", bufs=1, name=f"kb{R}")
            vb = pd.tile([NLOC_JT, 128, 520], f32, tag="vb", bufs=1, name=f"vb{R}")
            for ct in range(CT):
                nc.sync.dma_start(kb[ct, :, :], k[ct][:].bitcast(f32))
            for jt in range(NLOC_JT):
                nc.sync.dma_start(vb[jt, :, :], vT[jt][:].bitcast(f32))
            kg = pd.tile([2, CT, 128, NTOK], f32, tag="kg", bufs=1, name=f"kg{R}")
            _KG[0] = kg
            vg = pd.tile([2, NLOC_JT, 128, 520], f32, tag="vg", bufs=1, name=f"vg{R}")
            nc.gpsimd.collective_compute(
                "AllGather", ALU.bypass, replica_groups=pairs,
                ins=[kb[:].opt()], outs=[kg[:].opt()])
            nc.gpsimd.collective_compute(
                "AllGather", ALU.bypass, replica_groups=pairs,
                ins=[vb[:].opt()], outs=[vg[:].opt()])
            partner = (nc.partition_id() + 1) % 2
        else:
            kg = vg = partner = None

        s4sum = stats_cols(pa, "s4sum", 4)
        s4sq = stats_cols(pa, "s4sq", 4)
        scra = pa.tile([128, 512], f32, tag="scra", bufs=1, name=f"scra{R}")
        scl = float(1.0 / np.sqrt(C))

        for icn in range(4):
            sl = slice(icn * 512, (icn + 1) * 512)
            ps_o = [pp.tile([128, 512], f32, tag="acc_ps", bufs=4,
                            name=f"pso{R}_{icn}_{ct}") for ct in range(CT)]
            ps_den = pp.tile([1, 512], f32, tag="dn", bufs=1, name=f"psd{R}_{icn}")
            for jt in range(2 * NLOC_JT):
                if jt < NLOC_JT:
                    kt = [k[cit][:, jt * 128:(jt + 1) * 128] for cit in range(CT)]
                    vt = vT[jt][:]
                else:
                    jr = jt - NLOC_JT
                    kt = []
                    for cit in range(CT):
                        kr = pa.tile([128, 128], f32r, tag="krem", bufs=8,
                                     name=f"kr{R}_{icn}_{jr}_{cit}")
                        if pairs is not None:
                            nc.sync.dma_start(
                                kr[:], kg[bass.ds(partner, 1), cit, :,
                                          jr * 128:(jr + 1) * 128].bitcast(f32r))
                        else:
                            nc.vector.tensor_copy(kr[:], k[cit][:, jr * 128:(jr + 1) * 128].bitcast(f32))
                        kt.append(kr[:])
                    vr = pa.tile([128, 520], f32r, tag="vrem", bufs=2,
                                 name=f"vr{R}_{icn}_{jr}")
                    if pairs is not None:
                        nc.sync.dma_start(
                            vr[:], vg[bass.ds(partner, 1), jr, :, :].bitcast(f32r))
                    else:
                        nc.vector.tensor_copy(vr[:], vT[jr][:].bitcast(f32))
                    vt = vr[:]
                ps_s = pp.tile([128, 512], f32, tag="sc", bufs=3,
                               name=f"pss{R}_{icn}_{jt}")
                for cit in range(CT):
                    nc.tensor.matmul(ps_s[:], kt[cit], q[cit][:, sl],
                                     start=(cit == 0), stop=(cit == CT - 1))
                et = pa.tile([128, 512], f32r, tag="et", bufs=2,
                             name=f"et{R}_{icn}_{jt}")
                nc.scalar.activation(et[:], ps_s[:], AF.Exp, scale=scl)
                if dbg is not None and "yet" in dbg and icn == 0 and jt in (0, NLOC_JT):
                    nc.sync.dma_start(dbg["yet"][0 if jt == 0 else 1, :, :],
                                      et[:].bitcast(f32))
                st = (jt == 0)
                sp = (jt == 2 * NLOC_JT - 1)
                for cot in range(CT):
                    nc.tensor.matmul(ps_o[cot][:], vt[:, cot * 128:cot * 128 + 128],
                                     et[:], start=st, stop=sp)
                nc.tensor.matmul(ps_den[:], vt[:, C:C + 1], et[:], start=st, stop=sp)

            if dbg is not None and "yden" in dbg:
                dent = pa.tile([1, 512], f32, tag="dent", bufs=1, name=f"dent{R}_{icn}")
                nc.scalar.activation(dent[:], ps_den[:], AF.Copy)
                nc.sync.dma_start(dbg["yden"][icn, :, :], dent[:])
            recip = pa.tile([1, 512], f32, tag="recip", bufs=1, name=f"rc{R}_{icn}")
            nc.vector.reciprocal(recip[:], ps_den[:])
            ps_rb = pp.tile([128, 512], f32, tag="sc", bufs=3, name=f"psrb{R}_{icn}")
            nc.tensor.matmul(ps_rb[:], ones_r[:], recip[:], start=True, stop=True)
            rb = pa.tile([128, 512], f32, tag="rb", bufs=1, name=f"rb{R}_{icn}")
            nc.scalar.activation(rb[:], ps_rb[:], AF.Copy)
            on = []
            for cit in range(CT):
                o = pa.tile([128, 512], f32r, tag="on", bufs=CT,
                            name=f"on{R}_{icn}_{cit}")
                nc.vector.tensor_tensor(o[:], ps_o[cit][:], rb[:], op=ALU.mult)
                on.append(o)
            for cot in range(CT):
                ps_x = pp.tile([128, 512], f32, tag="sc", bufs=3,
                               name=f"psx{R}_{icn}_{cot}")
                for cit in range(CT):
                    nc.tensor.matmul(ps_x[:], wo[cit][:, cot * 128:cot * 128 + 128],
                                     on[cit][:], start=(cit == 0), stop=(cit == CT - 1))
                x2t = pa.tile([128, 512], f32, tag="x2t", bufs=2,
                              name=f"x2t{R}_{icn}_{cot}")
                nc.scalar.activation(x2t[:], ps_x[:], AF.Identity, bias=abo[cot][:])
                xr = pa.tile([128, 512], f32, tag="xr", bufs=2,
                             name=f"xr{R}_{icn}_{cot}")
                nc.sync.dma_start(xr[:], x1_dram[cot, :, sl])
                nc.vector.scalar_tensor_tensor(x2t[:], x2t[:], 1.0, xr[:],
                                               op0=ALU.mult, op1=ALU.add,
                                               accum_out=s4sum[cot][:, icn:icn + 1])
                nc.vector.scalar_tensor_tensor(scra[:], x2t[:], 1.0, x2t[:],
                                               op0=ALU.mult, op1=ALU.mult,
                                               accum_out=s4sq[cot][:, icn:icn + 1])
                nc.sync.dma_start(x2_dram[cot, :, sl], x2t[:])

        st4 = reduce_stats(pa, s4sum, s4sq, "st4")

        # boundary rows exchange for resnet2 (x2 rows 0,1,30,31)
        if pairs is not None:
            bb = pd.tile([CT, 128, 4, W], f32, tag="bb", bufs=1, name=f"bb{R}")
            for ct in range(CT):
                x2v = x2_dram[ct, :, :].rearrange("p (h w) -> p h w", w=W)
                nc.sync.dma_start(bb[ct, :, 0:2, :], x2v[:, 0:2, :])
                nc.sync.dma_start(bb[ct, :, 2:4, :], x2v[:, 30:32, :])
            bg = pd.tile([2, CT, 128, 4, W], f32, tag="bg", bufs=1, name=f"bg{R}")
            nc.gpsimd.collective_compute(
                "AllGather", ALU.bypass, replica_groups=pairs,
                ins=[bb[:].opt()], outs=[bg[:].opt()])
        else:
            bg = None
        A4, B4 = gn_finalize(pa, 3, st4)

    # =================== RESNET 2 ===================
    with tc.tile_pool(name=f"p2{R}", bufs=1) as p2:
        cb2 = load_bias(p2, cb_d[2], "cb2")
        cb3 = load_bias(p2, cb_d[3], "cb3")
        scr2 = [p2.tile([128, HS * W], f32, tag="scr2", bufs=2, name=f"sc2{R}_{i}")
                for i in range(2)]
        h3 = []
        for ct in range(CT):
            xt2 = p2.tile([128, NTOK], f32, tag="xt2", bufs=2, name=f"xt2{R}_{ct}")
            nc.sync.dma_start(xt2[:], x2_dram[ct, :, :])
            h = p2.tile([128, 36 * WP], f32r, tag="hp2", bufs=CT, name=f"h3{R}_{ct}")
            hr = h[:].rearrange("p (h w) -> p h w", w=WP)
            srcb = xt2[:, 0:36].rearrange("p (a b) -> p a b", a=36)
            nc.vector.tensor_scalar_mul(hr[:, :, 0:1], srcb, 0.0)
            nc.vector.tensor_scalar_mul(hr[:, :, WP - 1:WP], srcb, 0.0)
            xv = xt2[:].rearrange("p (h w) -> p h w", w=W)
            nc.scalar.activation(hr[:, 2:34, 1:WP - 1], xv, AF.Silu,
                                 bias=B4[ct][:], scale=A4[ct][:])
            # halo rows from partner x2 (masked to zero at image boundary)
            for (rr0, bslot, bc0, pmc) in ((0, 0, 2, 0), (34, 1, 0, 1)):
                hv = hr[:, rr0:rr0 + 2, 1:WP - 1]
                if bg is not None:
                    bt = p2.tile([128, 2 * W], f32, tag="bt", bufs=4,
                                 name=f"bt{R}_{ct}_{rr0}")
                    nc.sync.dma_start(bt[:], bg[bslot, ct, :, bc0:bc0 + 2, :])
                    src = bt[:].rearrange("p (h w) -> p h w", w=W)
                else:
                    src = xv[:, 0:2, :]
                nc.scalar.activation(hv, src, AF.Silu, bias=B4[ct][:], scale=A4[ct][:])
                nc.vector.tensor_scalar_mul(hv, hv, pm[:, pmc:pmc + 1])
            h3.append(h)

        s5sum = stats_cols(p2, "s5sum", len(ROWS34))
        s5sq = stats_cols(p2, "s5sq", len(ROWS34))

        def ep3(cot, r0, Rr, accT):
            a0, a1 = max(r0, 1), min(r0 + Rr, 33)
            v = accT[:, a0 * W:a1 * W]
            ci = ROWS34.index((r0, Rr))
            nc.scalar.activation(scr2[0][:, 0:(a1 - a0) * W], v, AF.Copy,
                                 accum_out=s5sum[cot][:, ci:ci + 1])
            nc.vector.scalar_tensor_tensor(scr2[1][:, 0:(a1 - a0) * W], v, 1.0, v,
                                           op0=ALU.mult, op1=ALU.mult,
                                           accum_out=s5sq[cot][:, ci:ci + 1])

        o3 = conv3x3(p2, h3, 34, ROWS34, "accB", 2, cb2, ep3)
        st5 = reduce_stats(p2, s5sum, s5sq, "st5")
        A5, B5 = gn_finalize(p2, 4, st5)

        h4 = []
        for ct in range(CT):
            h = p2.tile([128, 34 * WP], f32r, tag="hp2", bufs=CT, name=f"h4{R}_{ct}")
            hr = h[:].rearrange("p (h w) -> p h w", w=WP)
            srcb = o3[ct][:, 0:34].rearrange("p (a b) -> p a b", a=34)
            nc.vector.tensor_scalar_mul(hr[:, :, 0:1], srcb, 0.0)
            nc.vector.tensor_scalar_mul(hr[:, :, WP - 1:WP], srcb, 0.0)
            ov = o3[ct][:].rearrange("p (h w) -> p h w", w=W)
            nc.scalar.activation(hr[:, :, 1:WP - 1], ov, AF.Silu,
                                 bias=B5[ct][:], scale=A5[ct][:])
            nc.vector.tensor_scalar_mul(hr[:, 0:1, 1:WP - 1], hr[:, 0:1, 1:WP - 1],
                                        pm[:, 0:1])
            nc.vector.tensor_scalar_mul(hr[:, 33:34, 1:WP - 1], hr[:, 33:34, 1:WP - 1],
                                        pm[:, 1:2])
            h4.append(h)

        def ep4(cot, r0, Rr, accT):
            v = accT[:, r0 * W:(r0 + Rr) * W]
            xr2 = p2.tile([128, 512], f32, tag="xr2", bufs=3,
                          name=f"xr2{R}_{cot}_{r0}")
            nc.sync.dma_start(xr2[:, 0:Rr * W], x2_dram[cot, :, r0 * W:(r0 + Rr) * W])
            yt = p2.tile([128, 512], f32, tag="yt", bufs=3, name=f"yt{R}_{cot}_{r0}")
            nc.vector.tensor_tensor(yt[:, 0:Rr * W], v, xr2[:, 0:Rr * W], op=ALU.add)
            nc.sync.dma_start(y_d[cot * 128:(cot + 1) * 128, r0:r0 + Rr, :],
                              yt[:, 0:Rr * W])

        conv3x3(p2, h4, HS, ROWS32, "accB", 3, cb3, ep4)

    if dbg is not None:
        nc.sync.dma_start(dbg["y1"][:, :, :], x1_dram[:])
        nc.sync.dma_start(dbg["y2"][:, :, :], x2_dram[:])
        if "ykg" in dbg and pairs is not None:
            nc.sync.dma_start(dbg["ykg"][:, :, :, :], _KG[0][:])


# ======================= host side =======================

def _prep_inputs(inputs):
    x = inputs["x"]
    cw = np.stack([
        inputs["r1_w1"], inputs["r1_w2"], inputs["r2_w1"], inputs["r2_w2"]])
    # [conv, O=cot*128+co, I=cit*128+ci, ky, kx] ->
    # [conv, cit, ci, tap(ky*3+kx), cot, co]
    cwT = np.ascontiguousarray(
        cw.reshape(4, CT, 128, CT, 128, 3, 3)
          .transpose(0, 3, 4, 5, 6, 1, 2)
          .reshape(4, CT, 128, 9, CT, 128))
    cb = np.stack([inputs["r1_b1"], inputs["r1_b2"],
                   inputs["r2_b1"], inputs["r2_b2"]]).reshape(4, CT, 128)
    gn = np.ascontiguousarray(np.stack([
        np.stack([inputs["r1_g1s"], inputs["r1_g1b"]], axis=-1),
        np.stack([inputs["r1_g2s"], inputs["r1_g2b"]], axis=-1),
        np.stack([inputs["a_ns"], inputs["a_nb"]], axis=-1),
        np.stack([inputs["r2_g1s"], inputs["r2_g1b"]], axis=-1),
        np.stack([inputs["r2_g2s"], inputs["r2_g2b"]], axis=-1),
    ]).reshape(5, CT, 128, 2))

    def wT(w):  # [O, I] -> lhsT layout [cit, ci, cot*128+co]
        return w.reshape(CT, 128, CT, 128).transpose(2, 3, 0, 1).reshape(CT, 128, C)
    wqko = np.ascontiguousarray(
        np.stack([wT(inputs["a_wq"]), wT(inputs["a_wk"]), wT(inputs["a_wo"])]))
    wv = np.ascontiguousarray(inputs["a_wv"].T.reshape(CT, 128, C))
    ab = np.stack([inputs["a_bq"], inputs["a_bk"], inputs["a_bo"]]).reshape(3, CT, 128)
    bvb = np.ascontiguousarray(np.broadcast_to(inputs["a_bv"][None, :], (128, C)))
    ch = np.arange(C)
    gmask = (ch[:, None] // 16 == np.arange(G)[None, :]).astype(np.float32)
    gmaskT = np.ascontiguousarray(gmask.reshape(CT, 128, G))
    bmaskT = np.ascontiguousarray(gmask.T.reshape(G, CT, 128).transpose(1, 0, 2))

    in_maps = []
    for c in range(NCORES):
        b, h = c // 2, c % 2
        xpad = np.zeros((C, 36, W), np.float32)
        r0 = 32 * h - 2
        s0, s1 = max(r0, 0), min(r0 + 36, 64)
        xpad[:, s0 - r0:s1 - r0, :] = x[b, :, s0:s1, :]
        pmv = np.zeros((128, 2), np.float32)
        pmv[:, 0] = 1.0 if h == 1 else 0.0
        pmv[:, 1] = 1.0 if h == 0 else 0.0
        in_maps.append({
            "xpad": np.ascontiguousarray(xpad), "cw": cwT, "cb": cb, "gn": gn,
            "wqko": wqko, "wv": wv, "ab": ab, "bvb": bvb,
            "gmask": gmaskT, "bmask": bmaskT, "pm": pmv,
        })
    return in_maps


_nc_cache = {}


def _get_nc():
    if "nc" not in _nc_cache:
        _nc_cache["nc"] = build_midblock()
    return _nc_cache["nc"]


def kernel(**inputs):
    nc = _get_nc()
    in_maps = _prep_inputs(inputs)
    r = run_bass_kernel_spmd(nc, in_maps, list(range(NCORES)))
    out = np.empty((4, C, 64, W), np.float32)
    for c in range(NCORES):
        b, h = c // 2, c % 2
        out[b, :, 32 * h:32 * h + 32, :] = r.results[c]["y"]
    return out


# revision 13
# speedup vs baseline: 3.0848x; 3.0848x over previous
"""Trainium2 Bass kernel for nn_MidBlock (resnet -> attention -> resnet).

Sharding: 8 cores = (batch b, H-half h); core c handles batch c//2, rows
32*(c%2) .. +32.  GroupNorm stats pair-AllReduced; attention k/vT pair
AllGathered; conv halos computed redundantly from a host-padded input and a
boundary-row exchange before the second resnet.

Matmul dtype: float32r (1 cycle/row at N>=256, ~tf32 precision).
"""
import sys
sys.path.insert(0, '/opt/trn_rl_repo')
import numpy as np

import concourse.bass as bass
import concourse.bacc as bacc
import concourse.tile as tile
import concourse.mybir as mybir
from concourse.bass_utils import run_bass_kernel_spmd

f32 = mybir.dt.float32
f32r = mybir.dt.float32r
AF = mybir.ActivationFunctionType
ALU = mybir.AluOpType

NCORES = 8
PAIRS = [[0, 1], [2, 3], [4, 5], [6, 7]]
C = 512
CT = 4          # channel tiles of 128
G = 32          # groups
W = 64
WP = 66
HS = 32         # owned rows per core
NTOK = HS * W   # 2048 local tokens
NLOC_JT = NTOK // 128   # 16
EPS = 1e-5
GN_N = 16 * 64 * 64     # elements per group per batch
ROWS34 = [(0, 7), (7, 7), (14, 7), (21, 7), (28, 6)]
ROWS32 = [(0, 8), (8, 8), (16, 8), (24, 8)]
_KG = [None]


def build_midblock(num_devices=NCORES, collectives=True, reps=1, debug_outs=False):
    nc = bacc.Bacc("TRN2", target_bir_lowering=False, debug=False,
                   num_devices=num_devices)
    pairs = PAIRS if collectives else None

    xpad_d = nc.dram_tensor("xpad", [C, 36, W], f32, kind="ExternalInput")
    cw_d = nc.dram_tensor("cw", [4, CT, 128, 9, CT, 128], f32, kind="ExternalInput")
    cb_d = nc.dram_tensor("cb", [4, CT, 128], f32, kind="ExternalInput")
    gn_d = nc.dram_tensor("gn", [5, CT, 128, 2], f32, kind="ExternalInput")
    wqko_d = nc.dram_tensor("wqko", [3, CT, 128, CT * 128], f32, kind="ExternalInput")
    wv_d = nc.dram_tensor("wv", [CT, 128, C], f32, kind="ExternalInput")
    ab_d = nc.dram_tensor("ab", [3, CT, 128], f32, kind="ExternalInput")
    bvb_d = nc.dram_tensor("bvb", [128, C], f32, kind="ExternalInput")
    gmask_d = nc.dram_tensor("gmask", [CT, 128, G], f32, kind="ExternalInput")
    bmask_d = nc.dram_tensor("bmask", [CT, G, 128], f32, kind="ExternalInput")
    pm_d = nc.dram_tensor("pm", [128, 2], f32, kind="ExternalInput")
    y_d = nc.dram_tensor("y", [C, HS, W], f32, kind="ExternalOutput")
    dbg = None
    if debug_outs:
        dbg = {"y1": nc.dram_tensor("y1", [CT, 128, NTOK], f32, kind="ExternalOutput"),
               "y2": nc.dram_tensor("y2", [CT, 128, NTOK], f32, kind="ExternalOutput")}
        if debug_outs > 1:
            dbg["yq"] = nc.dram_tensor("yq", [CT, 128, NTOK], f32, kind="ExternalOutput")
            dbg["yk"] = nc.dram_tensor("yk", [CT, 128, NTOK], f32, kind="ExternalOutput")
            dbg["yv"] = nc.dram_tensor("yv", [NLOC_JT, 128, 520], f32, kind="ExternalOutput")
            dbg["ykg"] = nc.dram_tensor("ykg", [2, CT, 128, NTOK], f32, kind="ExternalOutput")
            dbg["yet"] = nc.dram_tensor("yet", [2, 128, 512], f32, kind="ExternalOutput")
            dbg["yden"] = nc.dram_tensor("yden", [4, 1, 512], f32, kind="ExternalOutput")
            dbg["yhn"] = nc.dram_tensor("yhn", [CT, 128, NTOK], f32, kind="ExternalOutput")
            dbg["yab"] = nc.dram_tensor("yab", [CT, 128, 2], f32, kind="ExternalOutput")

    with tile.TileContext(nc) as tc:
        with tc.tile_pool(name="pg", bufs=1) as pg, \
             tc.tile_pool(name="pp", bufs=1, space="PSUM") as pp, \
             tc.tile_pool(name="pd", bufs=1, space="DRAM") as pd:
            for rep in range(reps):
                _body(nc, tc, pg, pp, pd, pairs, rep,
                      xpad_d, cw_d, cb_d, gn_d, wqko_d, wv_d, ab_d, bvb_d,
                      gmask_d, bmask_d, pm_d, y_d, dbg)
    nc.compile()
    return nc


def _body(nc, tc, pg, pp, pd, pairs, rep,
          xpad_d, cw_d, cb_d, gn_d, wqko_d, wv_d, ab_d, bvb_d,
          gmask_d, bmask_d, pm_d, y_d, dbg=None):
    R = f"r{rep}"

    # ---------- global small tiles ----------
    gmask = []
    bmask = []
    for ct in range(CT):
        gm = pg.tile([128, G], f32, tag="gmask", bufs=CT, name=f"gm{R}_{ct}")
        nc.sync.dma_start(gm[:], gmask_d[ct, :, :])
        gmask.append(gm)
        bm = pg.tile([G, 128], f32, tag="bmask", bufs=CT, name=f"bm{R}_{ct}")
        nc.sync.dma_start(bm[:], bmask_d[ct, :, :])
        bmask.append(bm)
    pm = pg.tile([128, 2], f32, tag="pm", bufs=1, name=f"pm{R}")
    nc.sync.dma_start(pm[:], pm_d[:, :])
    ones_r = pg.tile([1, 128], f32, tag="ones_r", bufs=1, name=f"onr{R}")
    nc.vector.memset(ones_r[:], 1.0)

    def load_bias(pool, src_ap, tagn):
        out = []
        for ct in range(CT):
            b = pool.tile([128, 1], f32, tag=tagn, bufs=CT, name=f"{tagn}{R}_{ct}")
            nc.sync.dma_start(b[:], src_ap[ct, :])
            out.append(b)
        return out

    # preallocated per-channel (A, B) scale/bias tiles for all 5 GNs, so the
    # global pool never grows while a phase pool is open (fragmentation)
    AB = []
    for gi in range(5):
        Al = [pg.tile([128, 1], f32, tag=f"g{gi}A", bufs=CT, name=f"A{R}_{gi}_{c}")
              for c in range(CT)]
        Bl = [pg.tile([128, 1], f32, tag=f"g{gi}B", bufs=CT, name=f"B{R}_{gi}_{c}")
              for c in range(CT)]
        AB.append((Al, Bl))

    def gn_finalize(pool, gn_idx, stats2):
        """stats2: CT tiles [128,2] (sum, sumsq) per channel partition.
        Returns per-channel (A, B) scale/bias tiles [128,1] f32."""
        ps_g = pp.tile([G, 2], f32, tag="sc", bufs=3, name=f"psg{R}_{gn_idx}")
        for ct in range(CT):
            nc.tensor.matmul(ps_g[:], gmask[ct][:], stats2[ct][:],
                             start=(ct == 0), stop=(ct == CT - 1))
        sg = pool.tile([G, 2], f32, tag="sg", bufs=2, name=f"sg{R}_{gn_idx}")
        nc.scalar.activation(sg[:], ps_g[:], AF.Copy)
        sg2 = pool.tile([G, 2], f32, tag="sg2", bufs=2, name=f"sg2{R}_{gn_idx}")
        if pairs is not None:
            st_in = pd.tile([G, 2], f32, tag="st_in", bufs=2, name=f"sti{R}_{gn_idx}")
            st_out = pd.tile([G, 2], f32, tag="st_out", bufs=2, name=f"sto{R}_{gn_idx}")
            nc.sync.dma_start(st_in[:], sg[:])
            nc.gpsimd.collective_compute(
                "AllReduce", ALU.add, replica_groups=pairs,
                ins=[st_in[:].opt()], outs=[st_out[:].opt()])
            nc.sync.dma_start(sg2[:], st_out[:])
        else:
            nc.vector.tensor_copy(sg2[:], sg[:])
        mean2 = pool.tile([G, 2], f32, tag="mean2", bufs=2, name=f"mn{R}_{gn_idx}")
        nc.vector.tensor_scalar_mul(mean2[:], sg2[:], 1.0 / GN_N)
        var = pool.tile([G, 1], f32, tag="var", bufs=2, name=f"var{R}_{gn_idx}")
        nc.vector.scalar_tensor_tensor(var[:], mean2[:, 0:1], 1.0, mean2[:, 0:1],
                                       op0=ALU.mult, op1=ALU.mult)
        nc.vector.tensor_tensor(var[:], mean2[:, 1:2], var[:], op=ALU.subtract)
        nc.vector.tensor_scalar_add(var[:], var[:], EPS)
        sd = pool.tile([G, 1], f32, tag="sd", bufs=2, name=f"sd{R}_{gn_idx}")
        nc.scalar.activation(sd[:], var[:], AF.Sqrt)
        grp2 = pool.tile([G, 2], f32, tag="grp2", bufs=2, name=f"grp{R}_{gn_idx}")
        nc.vector.reciprocal(grp2[:, 0:1], sd[:])
        nc.vector.tensor_tensor(grp2[:, 1:2], mean2[:, 0:1], grp2[:, 0:1], op=ALU.mult)
        nc.vector.tensor_scalar_mul(grp2[:, 1:2], grp2[:, 1:2], -1.0)
        A, B = AB[gn_idx]
        for ct in range(CT):
            gnp = pool.tile([128, 2], f32, tag="gnp", bufs=2 * CT,
                            name=f"gnp{R}_{gn_idx}_{ct}")
            nc.sync.dma_start(gnp[:], gn_d[gn_idx, ct, :, :])
            ps_b = pp.tile([128, 2], f32, tag="sc", bufs=3, name=f"psb{R}_{gn_idx}_{ct}")
            nc.tensor.matmul(ps_b[:], bmask[ct][:], grp2[:], start=True, stop=True)
            bc = pool.tile([128, 2], f32, tag="bc", bufs=2 * CT,
                           name=f"bc{R}_{gn_idx}_{ct}")
            nc.scalar.activation(bc[:], ps_b[:], AF.Copy)
            nc.vector.tensor_tensor(A[ct][:], gnp[:, 0:1], bc[:, 0:1], op=ALU.mult)
            nc.vector.scalar_tensor_tensor(B[ct][:], gnp[:, 0:1], 1.0, bc[:, 1:2],
                                           op0=ALU.mult, op1=ALU.mult)
            nc.vector.tensor_tensor(B[ct][:], B[ct][:], gnp[:, 1:2], op=ALU.add)
        return A, B

    def conv3x3(pool, hbuf, nrows_out, rowplan, acc_tag, conv_idx, cb_tiles,
                epilogue):
        acc = [pool.tile([128, 34 * W], f32, tag=acc_tag, bufs=CT,
                         name=f"acc{R}_{conv_idx}_{ct}") for ct in range(CT)]
        for cit in range(CT):
            wt = pool.tile([128, 9 * C], f32r, tag="cw", bufs=2,
                           name=f"cw{R}_{conv_idx}_{cit}")
            nc.sync.dma_start(wt[:], cw_d[conv_idx, cit, :, :, :, :].bitcast(f32r))
            hb = hbuf[cit][:].rearrange("p (h w) -> p h w", w=WP)
            for cot in range(CT):
                for (r0, Rr) in rowplan:
                    ps = pp.tile([128, 512], f32, tag="acc_ps", bufs=4,
                                 name=f"cps{R}_{conv_idx}_{cit}_{cot}_{r0}")
                    psv = ps[:, 0:Rr * W]
                    for tap in range(9):
                        dy, dx = tap // 3, tap % 3
                        rhs = hb[:, r0 + dy:r0 + dy + Rr, dx:dx + W]
                        nc.tensor.matmul(
                            psv, wt[:, tap * C + cot * 128:tap * C + cot * 128 + 128],
                            rhs, start=(tap == 0), stop=(tap == 8))
                    accv = acc[cot][:, r0 * W:(r0 + Rr) * W]
                    if cit == 0:
                        nc.scalar.activation(accv, psv, AF.Identity, bias=cb_tiles[cot][:])
                    else:
                        nc.vector.tensor_tensor(accv, accv, psv, op=ALU.add)
                    if cit == CT - 1 and epilogue is not None:
                        epilogue(cot, r0, Rr, acc[cot])
        return acc

    def stats_cols(pool, tagn, ncols):
        return [pool.tile([128, ncols], f32, tag=tagn, bufs=CT,
                          name=f"{tagn}{R}_{ct}") for ct in range(CT)]

    def reduce_stats(pool, sumc, sqc, tagn):
        out = []
        for ct in range(CT):
            s2 = pool.tile([128, 2], f32, tag=tagn, bufs=CT, name=f"{tagn}{R}_{ct}")
            nc.vector.reduce_sum(s2[:, 0:1], sumc[ct][:], axis=mybir.AxisListType.X)
            nc.vector.reduce_sum(s2[:, 1:2], sqc[ct][:], axis=mybir.AxisListType.X)
            out.append(s2)
        return out

    # DRAM spill tensors
    x1_dram = pd.tile([CT, 128, NTOK], f32, tag="x1d", bufs=1, name=f"x1d{R}")
    x2_dram = pd.tile([CT, 128, NTOK], f32, tag="x2d", bufs=1, name=f"x2d{R}")

    # =================== RESNET 1 ===================
    with tc.tile_pool(name=f"p1{R}", bufs=1) as p1:
        cb0 = load_bias(p1, cb_d[0], "cb0")
        cb1 = load_bias(p1, cb_d[1], "cb1")
        scr = [p1.tile([128, HS * W], f32, tag="scr", bufs=2, name=f"scr{R}_{i}")
               for i in range(2)]
        xp = []
        s1sum = stats_cols(p1, "s1sum", 1)
        s1sq = stats_cols(p1, "s1sq", 1)
        for ct in range(CT):
            x = p1.tile([128, 36 * W], f32, tag="xp", bufs=CT, name=f"xp{R}_{ct}")
            nc.sync.dma_start(x[:], xpad_d[ct * 128:(ct + 1) * 128, :, :])
            xp.append(x)
            own = x[:, 2 * W:34 * W]
            nc.scalar.activation(scr[0][:], own, AF.Copy, accum_out=s1sum[ct][:, 0:1])
            nc.vector.scalar_tensor_tensor(scr[1][:], own, 1.0, own, op0=ALU.mult,
                                           op1=ALU.mult, accum_out=s1sq[ct][:, 0:1])
        st1 = reduce_stats(p1, s1sum, s1sq, "st1")
        A1, B1 = gn_finalize(p1, 0, st1)

        h1 = []
        for ct in range(CT):
            h = p1.tile([128, 36 * WP], f32r, tag="hp", bufs=CT, name=f"h1{R}_{ct}")
            hr = h[:].rearrange("p (h w) -> p h w", w=WP)
            srcb = xp[ct][:, 0:36].rearrange("p (a b) -> p a b", a=36)
            nc.vector.tensor_scalar_mul(hr[:, :, 0:1], srcb, 0.0)
            nc.vector.tensor_scalar_mul(hr[:, :, WP - 1:WP], srcb, 0.0)
            xv = xp[ct][:].rearrange("p (h w) -> p h w", w=W)
            nc.scalar.activation(hr[:, :, 1:WP - 1], xv, AF.Silu,
                                 bias=B1[ct][:], scale=A1[ct][:])
            # zero image-boundary halo rows (top unless odd core, bottom unless even)
            nc.vector.tensor_scalar_mul(hr[:, 0:2, 1:WP - 1], hr[:, 0:2, 1:WP - 1],
                                        pm[:, 0:1])
            nc.vector.tensor_scalar_mul(hr[:, 34:36, 1:WP - 1], hr[:, 34:36, 1:WP - 1],
                                        pm[:, 1:2])
            h1.append(h)

        s2sum = stats_cols(p1, "s2sum", len(ROWS34))
        s2sq = stats_cols(p1, "s2sq", len(ROWS34))

        def ep1(cot, r0, Rr, accT):
            a0, a1 = max(r0, 1), min(r0 + Rr, 33)   # owned rows of the 34-row grid
            v = accT[:, a0 * W:a1 * W]
            ci = ROWS34.index((r0, Rr))
            nc.scalar.activation(scr[0][:, 0:(a1 - a0) * W], v, AF.Copy,
                                 accum_out=s2sum[cot][:, ci:ci + 1])
            nc.vector.scalar_tensor_tensor(scr[1][:, 0:(a1 - a0) * W], v, 1.0, v,
                                           op0=ALU.mult, op1=ALU.mult,
                                           accum_out=s2sq[cot][:, ci:ci + 1])

        o1 = conv3x3(p1, h1, 34, ROWS34, "accA", 0, cb0, ep1)
        st2 = reduce_stats(p1, s2sum, s2sq, "st2")
        A2, B2 = gn_finalize(p1, 1, st2)

        h2 = []
        for ct in range(CT):
            h = p1.tile([128, 34 * WP], f32r, tag="hp", bufs=CT, name=f"h2{R}_{ct}")
            hr = h[:].rearrange("p (h w) -> p h w", w=WP)
            srcb = xp[ct][:, 0:34].rearrange("p (a b) -> p a b", a=34)
            nc.vector.tensor_scalar_mul(hr[:, :, 0:1], srcb, 0.0)
            nc.vector.tensor_scalar_mul(hr[:, :, WP - 1:WP], srcb, 0.0)
            ov = o1[ct][:].rearrange("p (h w) -> p h w", w=W)
            nc.scalar.activation(hr[:, :, 1:WP - 1], ov, AF.Silu,
                                 bias=B2[ct][:], scale=A2[ct][:])
            nc.vector.tensor_scalar_mul(hr[:, 0:1, 1:WP - 1], hr[:, 0:1, 1:WP - 1],
                                        pm[:, 0:1])
            nc.vector.tensor_scalar_mul(hr[:, 33:34, 1:WP - 1], hr[:, 33:34, 1:WP - 1],
                                        pm[:, 1:2])
            h2.append(h)

        s3sum = stats_cols(p1, "s3sum", len(ROWS32))
        s3sq = stats_cols(p1, "s3sq", len(ROWS32))

        def ep2(cot, r0, Rr, accT):
            ci = ROWS32.index((r0, Rr))
            v = accT[:, r0 * W:(r0 + Rr) * W]
            xv = xp[cot][:, (2 + r0) * W:(2 + r0 + Rr) * W]
            nc.vector.scalar_tensor_tensor(v, v, 1.0, xv, op0=ALU.mult, op1=ALU.add,
                                           accum_out=s3sum[cot][:, ci:ci + 1])
            nc.vector.scalar_tensor_tensor(scr[1][:, 0:Rr * W], v, 1.0, v,
                                           op0=ALU.mult, op1=ALU.mult,
                                           accum_out=s3sq[cot][:, ci:ci + 1])
            nc.sync.dma_start(x1_dram[cot, :, r0 * W:(r0 + Rr) * W], v)

        conv3x3(p1, h2, HS, ROWS32, "accA", 1, cb1, ep2)
        st3 = reduce_stats(p1, s3sum, s3sq, "st3")
        A3, B3 = gn_finalize(p1, 2, st3)

    # =================== ATTENTION ===================
    with tc.tile_pool(name=f"pa{R}", bufs=1) as pa:
        abq = load_bias(pa, ab_d[0], "abq")
        abk = load_bias(pa, ab_d[1], "abk")
        abo = load_bias(pa, ab_d[2], "abo")
        bvb = pa.tile([128, C], f32, tag="bvb", bufs=1, name=f"bvb{R}")
        nc.sync.dma_start(bvb[:], bvb_d[:, :])
        wo = []
        for ct in range(CT):
            wt = pa.tile([128, C], f32r, tag="wo", bufs=CT, name=f"wo{R}_{ct}")
            nc.sync.dma_start(wt[:], wqko_d[2, ct, :, :].bitcast(f32r))
            wo.append(wt)
        q = [pa.tile([128, NTOK], f32r, tag="q", bufs=CT, name=f"q{R}_{c}")
             for c in range(CT)]
        k = [pa.tile([128, NTOK], f32r, tag="k", bufs=CT, name=f"k{R}_{c}")
             for c in range(CT)]
        vT = [pa.tile([128, 520], f32r, tag="vt", bufs=NLOC_JT, name=f"vt{R}_{jt}")
              for jt in range(NLOC_JT)]

        with tc.tile_pool(name=f"pre{R}", bufs=1) as pre:
            hn = []
            for ct in range(CT):
                xt = pre.tile([128, NTOK], f32, tag="xt", bufs=1, name=f"xt{R}_{ct}")
                nc.sync.dma_start(xt[:], x1_dram[ct, :, :])
                h = pa.tile([128, NTOK], f32r, tag="hn", bufs=CT, name=f"hn{R}_{ct}")
                # NOTE: the attention block's GroupNorm has NO activation
                nc.scalar.activation(h[:], xt[:], AF.Identity,
                                     bias=B3[ct][:], scale=A3[ct][:])
                hn.append(h)
                if dbg is not None and "yhn" in dbg:
                    nc.sync.dma_start(dbg["yhn"][ct, :, :], h[:].bitcast(f32))
                    nc.sync.dma_start(dbg["yab"][ct, :, 0:1], A3[ct][:])
                    nc.sync.dma_start(dbg["yab"][ct, :, 1:2], B3[ct][:])

            for idx, (lst, bias) in enumerate(((q, abq), (k, abk))):
                wts = []
                for ct in range(CT):
                    wt = pre.tile([128, C], f32r, tag="aw", bufs=CT,
                                  name=f"aw{R}_{idx}_{ct}")
                    nc.sync.dma_start(wt[:], wqko_d[idx, ct, :, :].bitcast(f32r))
                    wts.append(wt)
                for cot in range(CT):
                    for icn in range(4):
                        sl = slice(icn * 512, (icn + 1) * 512)
                        ps = pp.tile([128, 512], f32, tag="sc", bufs=3,
                                     name=f"psqk{R}_{idx}_{cot}_{icn}")
                        for cit in range(CT):
                            nc.tensor.matmul(
                                ps[:], wts[cit][:, cot * 128:cot * 128 + 128],
                                hn[cit][:, sl],
                                start=(cit == 0), stop=(cit == CT - 1))
                        nc.scalar.activation(lst[cot][:, sl], ps[:], AF.Identity,
                                             bias=bias[cot][:])

            wv = []
            for ct in range(CT):
                wt = pre.tile([128, C], f32r, tag="aw", bufs=CT,
                              name=f"awv{R}_{ct}")
                nc.sync.dma_start(wt[:], wv_d[ct, :, :].bitcast(f32r))
                wv.append(wt)
            for jt in range(NLOC_JT):
                ps = pp.tile([128, 512], f32, tag="sc", bufs=3, name=f"psv{R}_{jt}")
                for cit in range(CT):
                    nc.tensor.matmul(ps[:], hn[cit][:, jt * 128:(jt + 1) * 128],
                                     wv[cit][:], start=(cit == 0), stop=(cit == CT - 1))
                t = vT[jt]
                nc.scalar.activation(t[:, 0:C], ps[:], AF.Copy)
                nc.vector.tensor_tensor(t[:, 0:C], t[:, 0:C], bvb[:], op=ALU.add)
                nc.vector.tensor_scalar(t[:, C:C + 1], bvb[:, 0:1], 0.0, 1.0,
                                        op0=ALU.mult, op1=ALU.add)

        if dbg is not None and "yq" in dbg:
            for ct in range(CT):
                nc.sync.dma_start(dbg["yq"][ct, :, :], q[ct][:].bitcast(f32))
                nc.sync.dma_start(dbg["yk"][ct, :, :], k[ct][:].bitcast(f32))
            for jt in range(NLOC_JT):
                nc.sync.dma_start(dbg["yv"][jt, :, :], vT[jt][:].bitcast(f32))
        # exchange k / vT with pair partner
        if pairs is not None:
            kb = pd.tile([CT, 128, NTOK], f32, tag="kb", bufs=1, name=f"kb{R}")
            vb = pd.tile([NLOC_JT, 128, 520], f32, tag="vb", bufs=1, name=f"vb{R}")
            for ct in range(CT):
                nc.sync.dma_start(kb[ct, :, :], k[ct][:].bitcast(f32))
            for jt in range(NLOC_JT):
                nc.sync.dma_start(vb[jt, :, :], vT[jt][:].bitcast(f32))
            kg = pd.tile([2, CT, 128, NTOK], f32, tag="kg", bufs=1, name=f"kg{R}")
            _KG[0] = kg
            vg = pd.tile([2, NLOC_JT, 128, 520], f32, tag="vg", bufs=1, name=f"vg{R}")
            nc.gpsimd.collective_compute(
                "AllGather", ALU.bypass, replica_groups=pairs,
                ins=[kb[:].opt()], outs=[kg[:].opt()])
            nc.gpsimd.collective_compute(
                "AllGather", ALU.bypass, replica_groups=pairs,
                ins=[vb[:].opt()], outs=[vg[:].opt()])
            partner = (nc.partition_id() + 1) % 2
        else:
            kg = vg = partner = None

        krem = []
        for cit in range(CT):
            krt = pa.tile([128, NTOK], f32r, tag="hn", bufs=CT,
                          name=f"krt{R}_{cit}")
            if pairs is not None:
                nc.sync.dma_start(krt[:], kg[bass.ds(partner, 1), cit, :, :].bitcast(f32r))
            else:
                nc.vector.tensor_copy(krt[:], k[cit][:].bitcast(f32))
            krem.append(krt)

        s4sum = stats_cols(pa, "s4sum", 4)
        s4sq = stats_cols(pa, "s4sq", 4)
        scra = pa.tile([128, 512], f32, tag="scra", bufs=1, name=f"scra{R}")
        scl = float(1.0 / np.sqrt(C))

        for icn in range(4):
            sl = slice(icn * 512, (icn + 1) * 512)
            ps_o = [pp.tile([128, 512], f32, tag="acc_ps", bufs=4,
                            name=f"pso{R}_{icn}_{ct}") for ct in range(CT)]
            ps_den = pp.tile([1, 512], f32, tag="dn", bufs=1, name=f"psd{R}_{icn}")
            for jt in range(2 * NLOC_JT):
                if jt < NLOC_JT:
                    kt = [k[cit][:, jt * 128:(jt + 1) * 128] for cit in range(CT)]
                    vt = vT[jt][:]
                else:
                    jr = jt - NLOC_JT
                    kt = [krem[cit][:, jr * 128:(jr + 1) * 128] for cit in range(CT)]
                    vr = pa.tile([128, 520], f32r, tag="vrem", bufs=2,
                                 name=f"vr{R}_{icn}_{jr}")
                    if pairs is not None:
                        nc.sync.dma_start(
                            vr[:], vg[bass.ds(partner, 1), jr, :, :].bitcast(f32r))
                    else:
                        nc.vector.tensor_copy(vr[:], vT[jr][:].bitcast(f32))
                    vt = vr[:]
                ps_s = pp.tile([128, 512], f32, tag="sc", bufs=3,
                               name=f"pss{R}_{icn}_{jt}")
                for cit in range(CT):
                    nc.tensor.matmul(ps_s[:], kt[cit], q[cit][:, sl],
                                     start=(cit == 0), stop=(cit == CT - 1))
                et = pa.tile([128, 512], f32r, tag="et", bufs=2,
                             name=f"et{R}_{icn}_{jt}")
                nc.scalar.activation(et[:], ps_s[:], AF.Exp, scale=scl)
                if dbg is not None and "yet" in dbg and icn == 0 and jt in (0, NLOC_JT):
                    nc.sync.dma_start(dbg["yet"][0 if jt == 0 else 1, :, :],
                                      et[:].bitcast(f32))
                st = (jt == 0)
                sp = (jt == 2 * NLOC_JT - 1)
                for cot in range(CT):
                    nc.tensor.matmul(ps_o[cot][:], vt[:, cot * 128:cot * 128 + 128],
                                     et[:], start=st, stop=sp)
                nc.tensor.matmul(ps_den[:], vt[:, C:C + 1], et[:], start=st, stop=sp)

            if dbg is not None and "yden" in dbg:
                dent = pa.tile([1, 512], f32, tag="dent", bufs=1, name=f"dent{R}_{icn}")
                nc.scalar.activation(dent[:], ps_den[:], AF.Copy)
                nc.sync.dma_start(dbg["yden"][icn, :, :], dent[:])
            recip = pa.tile([1, 512], f32, tag="recip", bufs=1, name=f"rc{R}_{icn}")
            nc.vector.reciprocal(recip[:], ps_den[:])
            ps_rb = pp.tile([128, 512], f32, tag="sc", bufs=3, name=f"psrb{R}_{icn}")
            nc.tensor.matmul(ps_rb[:], ones_r[:], recip[:], start=True, stop=True)
            rb = pa.tile([128, 512], f32, tag="rb", bufs=1, name=f"rb{R}_{icn}")
            nc.scalar.activation(rb[:], ps_rb[:], AF.Copy)
            on = []
            for cit in range(CT):
                o = pa.tile([128, 512], f32r, tag="on", bufs=CT,
                            name=f"on{R}_{icn}_{cit}")
                nc.vector.tensor_tensor(o[:], ps_o[cit][:], rb[:], op=ALU.mult)
                on.append(o)
            for cot in range(CT):
                ps_x = pp.tile([128, 512], f32, tag="sc", bufs=3,
                               name=f"psx{R}_{icn}_{cot}")
                for cit in range(CT):
                    nc.tensor.matmul(ps_x[:], wo[cit][:, cot * 128:cot * 128 + 128],
                                     on[cit][:], start=(cit == 0), stop=(cit == CT - 1))
                x2t = pa.tile([128, 512], f32, tag="x2t", bufs=2,
                              name=f"x2t{R}_{icn}_{cot}")
                nc.scalar.activation(x2t[:], ps_x[:], AF.Identity, bias=abo[cot][:])
                xr = pa.tile([128, 512], f32, tag="xr", bufs=2,
                             name=f"xr{R}_{icn}_{cot}")
                nc.sync.dma_start(xr[:], x1_dram[cot, :, sl])
                nc.vector.scalar_tensor_tensor(x2t[:], x2t[:], 1.0, xr[:],
                                               op0=ALU.mult, op1=ALU.add,
                                               accum_out=s4sum[cot][:, icn:icn + 1])
                nc.vector.scalar_tensor_tensor(scra[:], x2t[:], 1.0, x2t[:],
                                               op0=ALU.mult, op1=ALU.mult,
                                               accum_out=s4sq[cot][:, icn:icn + 1])
                nc.sync.dma_start(x2_dram[cot, :, sl], x2t[:])

        st4 = reduce_stats(pa, s4sum, s4sq, "st4")

        # boundary rows exchange for resnet2 (x2 rows 0,1,30,31)
        if pairs is not None:
            bb = pd.tile([CT, 128, 4, W], f32, tag="bb", bufs=1, name=f"bb{R}")
            for ct in range(CT):
                x2v = x2_dram[ct, :, :].rearrange("p (h w) -> p h w", w=W)
                nc.sync.dma_start(bb[ct, :, 0:2, :], x2v[:, 0:2, :])
                nc.sync.dma_start(bb[ct, :, 2:4, :], x2v[:, 30:32, :])
            bg = pd.tile([2, CT, 128, 4, W], f32, tag="bg", bufs=1, name=f"bg{R}")
            nc.gpsimd.collective_compute(
                "AllGather", ALU.bypass, replica_groups=pairs,
                ins=[bb[:].opt()], outs=[bg[:].opt()])
        else:
            bg = None
        A4, B4 = gn_finalize(pa, 3, st4)

    # =================== RESNET 2 ===================
    with tc.tile_pool(name=f"p2{R}", bufs=1) as p2:
        cb2 = load_bias(p2, cb_d[2], "cb2")
        cb3 = load_bias(p2, cb_d[3], "cb3")
        scr2 = [p2.tile([128, HS * W], f32, tag="scr2", bufs=2, name=f"sc2{R}_{i}")
                for i in range(2)]
        h3 = []
        for ct in range(CT):
            xt2 = p2.tile([128, NTOK], f32, tag="xt2", bufs=2, name=f"xt2{R}_{ct}")
            nc.sync.dma_start(xt2[:], x2_dram[ct, :, :])
            h = p2.tile([128, 36 * WP], f32r, tag="hp2", bufs=CT, name=f"h3{R}_{ct}")
            hr = h[:].rearrange("p (h w) -> p h w", w=WP)
            srcb = xt2[:, 0:36].rearrange("p (a b) -> p a b", a=36)
            nc.vector.tensor_scalar_mul(hr[:, :, 0:1], srcb, 0.0)
            nc.vector.tensor_scalar_mul(hr[:, :, WP - 1:WP], srcb, 0.0)
            xv = xt2[:].rearrange("p (h w) -> p h w", w=W)
            nc.scalar.activation(hr[:, 2:34, 1:WP - 1], xv, AF.Silu,
                                 bias=B4[ct][:], scale=A4[ct][:])
            # halo rows from partner x2 (masked to zero at image boundary)
            for (rr0, bslot, bc0, pmc) in ((0, 0, 2, 0), (34, 1, 0, 1)):
                hv = hr[:, rr0:rr0 + 2, 1:WP - 1]
                if bg is not None:
                    bt = p2.tile([128, 2 * W], f32, tag="bt", bufs=4,
                                 name=f"bt{R}_{ct}_{rr0}")
                    nc.sync.dma_start(bt[:], bg[bslot, ct, :, bc0:bc0 + 2, :])
                    src = bt[:].rearrange("p (h w) -> p h w", w=W)
                else:
                    src = xv[:, 0:2, :]
                nc.scalar.activation(hv, src, AF.Silu, bias=B4[ct][:], scale=A4[ct][:])
                nc.vector.tensor_scalar_mul(hv, hv, pm[:, pmc:pmc + 1])
            h3.append(h)

        s5sum = stats_cols(p2, "s5sum", len(ROWS34))
        s5sq = stats_cols(p2, "s5sq", len(ROWS34))

        def ep3(cot, r0, Rr, accT):
            a0, a1 = max(r0, 1), min(r0 + Rr, 33)
            v = accT[:, a0 * W:a1 * W]
            ci = ROWS34.index((r0, Rr))
            nc.scalar.activation(scr2[0][:, 0:(a1 - a0) * W], v, AF.Copy,
                                 accum_out=s5sum[cot][:, ci:ci + 1])
            nc.vector.scalar_tensor_tensor(scr2[1][:, 0:(a1 - a0) * W], v, 1.0, v,
                                           op0=ALU.mult, op1=ALU.mult,
                                           accum_out=s5sq[cot][:, ci:ci + 1])

        o3 = conv3x3(p2, h3, 34, ROWS34, "accB", 2, cb2, ep3)
        st5 = reduce_stats(p2, s5sum, s5sq, "st5")
        A5, B5 = gn_finalize(p2, 4, st5)

        h4 = []
        for ct in range(CT):
            h = p2.tile([128, 34 * WP], f32r, tag="hp2", bufs=CT, name=f"h4{R}_{ct}")
            hr = h[:].rearrange("p (h w) -> p h w", w=WP)
            srcb = o3[ct][:, 0:34].rearrange("p (a b) -> p a b", a=34)
            nc.vector.tensor_scalar_mul(hr[:, :, 0:1], srcb, 0.0)
            nc.vector.tensor_scalar_mul(hr[:, :, WP - 1:WP], srcb, 0.0)
            ov = o3[ct][:].rearrange("p (h w) -> p h w", w=W)
            nc.scalar.activation(hr[:, :, 1:WP - 1], ov, AF.Silu,
                                 bias=B5[ct][:], scale=A5[ct][:])
            nc.vector.tensor_scalar_mul(hr[:, 0:1, 1:WP - 1], hr[:, 0:1, 1:WP - 1],
                                        pm[:, 0:1])
            nc.vector.tensor_scalar_mul(hr[:, 33:34, 1:WP - 1], hr[:, 33:34, 1:WP - 1],
                                        pm[:, 1:2])
            h4.append(h)

        def ep4(cot, r0, Rr, accT):
            v = accT[:, r0 * W:(r0 + Rr) * W]
            xr2 = p2.tile([128, 512], f32, tag="xr2", bufs=3,
                          name=f"xr2{R}_{cot}_{r0}")
            nc.sync.dma_start(xr2[:, 0:Rr * W], x2_dram[cot, :, r0 * W:(r0 + Rr) * W])
            yt = p2.tile([128, 512], f32, tag="yt", bufs=3, name=f"yt{R}_{cot}_{r0}")
            nc.vector.tensor_tensor(yt[:, 0:Rr * W], v, xr2[:, 0:Rr * W], op=ALU.add)
            nc.sync.dma_start(y_d[cot * 128:(cot + 1) * 128, r0:r0 + Rr, :],
                              yt[:, 0:Rr * W])

        conv3x3(p2, h4, HS, ROWS32, "accB", 3, cb3, ep4)

    if dbg is not None:
        nc.sync.dma_start(dbg["y1"][:, :, :], x1_dram[:])
        nc.sync.dma_start(dbg["y2"][:, :, :], x2_dram[:])
        if "ykg" in dbg and pairs is not None:
            nc.sync.dma_start(dbg["ykg"][:, :, :, :], _KG[0][:])


# ======================= host side =======================

def _prep_inputs(inputs):
    x = inputs["x"]
    cw = np.stack([
        inputs["r1_w1"], inputs["r1_w2"], inputs["r2_w1"], inputs["r2_w2"]])
    # [conv, O=cot*128+co, I=cit*128+ci, ky, kx] ->
    # [conv, cit, ci, tap(ky*3+kx), cot, co]
    cwT = np.ascontiguousarray(
        cw.reshape(4, CT, 128, CT, 128, 3, 3)
          .transpose(0, 3, 4, 5, 6, 1, 2)
          .reshape(4, CT, 128, 9, CT, 128))
    cb = np.stack([inputs["r1_b1"], inputs["r1_b2"],
                   inputs["r2_b1"], inputs["r2_b2"]]).reshape(4, CT, 128)
    gn = np.ascontiguousarray(np.stack([
        np.stack([inputs["r1_g1s"], inputs["r1_g1b"]], axis=-1),
        np.stack([inputs["r1_g2s"], inputs["r1_g2b"]], axis=-1),
        np.stack([inputs["a_ns"], inputs["a_nb"]], axis=-1),
        np.stack([inputs["r2_g1s"], inputs["r2_g1b"]], axis=-1),
        np.stack([inputs["r2_g2s"], inputs["r2_g2b"]], axis=-1),
    ]).reshape(5, CT, 128, 2))

    def wT(w):  # [O, I] -> lhsT layout [cit, ci, cot*128+co]
        return w.reshape(CT, 128, CT, 128).transpose(2, 3, 0, 1).reshape(CT, 128, C)
    wqko = np.ascontiguousarray(
        np.stack([wT(inputs["a_wq"]), wT(inputs["a_wk"]), wT(inputs["a_wo"])]))
    wv = np.ascontiguousarray(inputs["a_wv"].T.reshape(CT, 128, C))
    ab = np.stack([inputs["a_bq"], inputs["a_bk"], inputs["a_bo"]]).reshape(3, CT, 128)
    bvb = np.ascontiguousarray(np.broadcast_to(inputs["a_bv"][None, :], (128, C)))
    ch = np.arange(C)
    gmask = (ch[:, None] // 16 == np.arange(G)[None, :]).astype(np.float32)
    gmaskT = np.ascontiguousarray(gmask.reshape(CT, 128, G))
    bmaskT = np.ascontiguousarray(gmask.T.reshape(G, CT, 128).transpose(1, 0, 2))

    in_maps = []
    for c in range(NCORES):
        b, h = c // 2, c % 2
        xpad = np.zeros((C, 36, W), np.float32)
        r0 = 32 * h - 2
        s0, s1 = max(r0, 0), min(r0 + 36, 64)
        xpad[:, s0 - r0:s1 - r0, :] = x[b, :, s0:s1, :]
        pmv = np.zeros((128, 2), np.float32)
        pmv[:, 0] = 1.0 if h == 1 else 0.0
        pmv[:, 1] = 1.0 if h == 0 else 0.0
        in_maps.append({
            "xpad": np.ascontiguousarray(xpad), "cw": cwT, "cb": cb, "gn": gn,
            "wqko": wqko, "wv": wv, "ab": ab, "bvb": bvb,
            "gmask": gmaskT, "bmask": bmaskT, "pm": pmv,
        })
    return in_maps


_nc_cache = {}


def _get_nc():
    if "nc" not in _nc_cache:
        _nc_cache["nc"] = build_midblock()
    return _nc_cache["nc"]


def kernel(**inputs):
    nc = _get_nc()
    in_maps = _prep_inputs(inputs)
    r = run_bass_kernel_spmd(nc, in_maps, list(range(NCORES)))
    out = np.empty((4, C, 64, W), np.float32)
    for c in range(NCORES):
        b, h = c // 2, c % 2
        out[b, :, 32 * h:32 * h + 32, :] = r.results[c]["y"]
    return out
